# revision 29
# baseline (speedup 1.0000x reference)
"""Trainium2 Bass kernel: top-k cosine retrieval adjacency + TF-IDF block.

Builds the dense (N+P+V)^2 adjacency and node features of the reference
nn.Module on 8 NeuronCores.

Work decomposition (SPMD, identical program on all 8 cores, data-parallel):
  - core c computes the top-4 one-hot rows for x rows [512c, 512(c+1))
  - core c computes TF-IDF block rows [64c, 64(c+1))
  - core c writes 1/8 of the zero canvas rows of the adjacency
  - node_feat is a passthrough copy, sharded the same way
Host side only re-assembles the row shards (np.concatenate) into the
full [8704, 8704] adjacency and [4608, 256] node features.
"""

import numpy as np

N, P, V, H, TOPK = 4096, 512, 4096, 256, 4
DIM = N + P + V            # 8704
NCORES = 8
NX = N // NCORES           # 512 x rows per core
NB = P // NCORES           # 64 prototype rows per core
ZR = DIM // NCORES         # 1088 canvas rows per core

# device writes the full zero canvas (memory-roofline path). If False the
# canvas relies on the runtime's zero-initialized output buffers.
WRITE_ZEROS = True
# shard the document-frequency column sums across cores + AllGather instead of
# every core reading the full replicated prototype_count.
USE_ALLGATHER = False

_NC_CACHE = {}


def _emit(tc, aps, it=0, write_zeros=True, zero_bcast=True, use_allgather=False):
    """Emit the per-core program into TileContext tc."""
    from contextlib import ExitStack

    import concourse.mybir as mybir
    from concourse.masks import make_identity

    nc = tc.nc
    fp32 = mybir.dt.float32
    bf16 = mybir.dt.bfloat16
    AF = mybir.ActivationFunctionType
    OP = mybir.AluOpType
    AX = mybir.AxisListType

    t_xs = aps["xs"]
    t_vq = aps["vq"]
    t_pc = aps.get("pc")
    t_pcr = aps["pcr"]
    t_vqs = aps["vqs"]
    t_oh = aps["onehot"]
    t_blk = aps["block"]
    t_zer = aps["zeros"]
    t_nfx = aps["nfx"]
    t_nfk = aps["nfk"]

    with ExitStack() as ctx:
        # the full-width zero tile (zero_bcast=False) needs the SBUF that the
        # 4-deep pc streaming pool otherwise uses
        iop_bufs = 4 if (use_allgather or zero_bcast) else 2
        res = ctx.enter_context(tc.tile_pool(name=f"res{it}", bufs=1))
        iop = ctx.enter_context(tc.tile_pool(name=f"iop{it}", bufs=iop_bufs))
        work = ctx.enter_context(tc.tile_pool(name=f"work{it}", bufs=2))
        lnp = ctx.enter_context(tc.tile_pool(name=f"lnp{it}", bufs=1))
        pst = ctx.enter_context(tc.tile_pool(name=f"pst{it}", bufs=2, space="PSUM"))
        pss = ctx.enter_context(tc.tile_pool(name=f"pss{it}", bufs=2, space="PSUM"))
        psc = ctx.enter_context(tc.tile_pool(name=f"psc{it}", bufs=1, space="PSUM"))

        # ----- input loads first so the compute chain starts immediately and
        # its (small) stores hide under the bulk zero-canvas writes -----
        xt, vqt, knt = [], [], []
        for t in range(4):
            xtile = res.tile([128, H], fp32, tag=f"x{t}")
            nc.scalar.dma_start(out=xtile[:], in_=t_xs[128 * t : 128 * (t + 1), :])
            xt.append(xtile)
        for t in range(4):
            vtile = res.tile([128, H], fp32, tag=f"vq{t}")
            nc.scalar.dma_start(out=vtile[:], in_=t_vq[128 * t : 128 * (t + 1), :])
            vqt.append(vtile)
        pcrt = res.tile([NB, V], fp32, tag="pcr")
        nc.scalar.dma_start(out=pcrt[:], in_=t_pcr[:, :])
        vqst = res.tile([NB, H], fp32, tag="vqs")
        nc.scalar.dma_start(out=vqst[:], in_=t_vqs[:, :])
        pcts = []
        if use_allgather:
            # per-core column slice of prototype_count: [P, V/NCORES]
            t_pcc = aps["pcc"]
            for k in range(4):
                pct = iop.tile([128, V // NCORES], fp32, tag="pc", name=f"pct{k}")
                nc.scalar.dma_start(out=pct[:], in_=t_pcc[128 * k : 128 * (k + 1), :])
                pcts.append(pct)
        else:
            for k in range(4):
                pct = iop.tile([128, V], fp32, tag="pc", name=f"pct{k}")
                nc.scalar.dma_start(out=pct[:], in_=t_pc[128 * k : 128 * (k + 1), :])
                pcts.append(pct)

        # ----- zero canvas: the bulk HBM write (ZR x DIM f32 per core) -----
        if write_zeros:
            nzt = 9  # 1088 rows = 8*128 + 64
            if zero_bcast == "mega":
                # two DMAs total: [1024, DIM] + [64, DIM], broadcast source
                zt = res.tile([128, 512], fp32, tag="zt")
                nc.vector.memset(zt[:], 0.0)
                reps = DIM // 512  # 17
                src = zt[:, :].unsqueeze(1).to_broadcast((128, 8 * reps, 512))
                nc.sync.dma_start(out=t_zer[0:1024, :], in_=src)
                src2 = zt[:64, :].unsqueeze(1).to_broadcast((64, reps, 512))
                nc.scalar.dma_start(out=t_zer[1024:ZR, :], in_=src2)
            elif zero_bcast:
                zt = res.tile([128, 512], fp32, tag="zt")
                nc.vector.memset(zt[:], 0.0)
                reps = DIM // 512  # 17
                for i in range(nzt):
                    r0 = 128 * i
                    r1 = min(ZR, r0 + 128)
                    rows = r1 - r0
                    src = zt[:rows, :].unsqueeze(1).to_broadcast((rows, reps, 512))
                    eng = nc.sync if i in (0, 2, 3, 5, 7, 8) else nc.scalar
                    eng.dma_start(out=t_zer[r0:r1, :], in_=src)
            else:
                zt = res.tile([128, DIM], fp32, tag="zt")
                nc.vector.memset(zt[:], 0.0)
                for i in range(nzt):
                    r0 = 128 * i
                    r1 = min(ZR, r0 + 128)
                    eng = nc.sync if i in (0, 2, 3, 5, 7, 8) else nc.scalar
                    eng.dma_start(out=t_zer[r0:r1, :], in_=zt[: r1 - r0, :])

        # ----- normalize keys -----
        for t in range(4):
            sq = work.tile([128, H], fp32, tag="sq")
            ss = res.tile([128, 1], fp32, tag=f"ss{t}")
            nc.scalar.activation(out=sq[:], in_=vqt[t][:], func=AF.Square, accum_out=ss[:])
            nrm = res.tile([128, 1], fp32, tag=f"nrm{t}")
            nc.scalar.sqrt(nrm[:], ss[:])
            rn = res.tile([128, 1], fp32, tag=f"rn{t}")
            nc.vector.reciprocal(rn[:], nrm[:])
            kt = res.tile([128, H], fp32, tag=f"kn{t}")
            nc.vector.tensor_scalar_mul(kt[:], vqt[t][:], rn[:])
            knt.append(kt)

        # ----- PE transposes: x^T [H, NX], kn^T [H, P] -----
        ident = res.tile([128, 128], fp32, tag="ident")
        make_identity(nc, ident[:])
        # warmup: advance PE's observed clock past the gpsimd identity write so
        # real transposes carry a single semaphore wait (walrus LDW limit).
        warm = pst.tile([128, 128], fp32, tag="tp", name="warm")
        nc.tensor.transpose(warm[:], ident[:], ident[:])
        xT = [res.tile([128, NX], fp32, tag=f"xT{h}", name=f"xT{h}") for h in range(2)]
        kT = [res.tile([128, P], fp32, tag=f"kT{h}", name=f"kT{h}") for h in range(2)]
        for t in range(4):
            for h in range(2):
                ps = pst.tile([128, 128], fp32, tag="tp")
                nc.tensor.transpose(ps[:], xt[t][:, 128 * h : 128 * (h + 1)], ident[:])
                nc.vector.tensor_copy(xT[h][:, 128 * t : 128 * (t + 1)], ps[:])
                ps2 = pst.tile([128, 128], fp32, tag="tp")
                nc.tensor.transpose(ps2[:], knt[t][:, 128 * h : 128 * (h + 1)], ident[:])
                nc.vector.tensor_copy(kT[h][:, 128 * t : 128 * (t + 1)], ps2[:])

        # ----- s = x @ kn^T (f32), top-4 -> one-hot rows -----
        for t in range(4):
            ps_s = pss.tile([128, P], fp32, tag="s")
            nc.tensor.matmul(
                out=ps_s[:], lhsT=xT[0][:, 128 * t : 128 * (t + 1)], rhs=kT[0][:],
                start=True, stop=False,
            )
            nc.tensor.matmul(
                out=ps_s[:], lhsT=xT[1][:, 128 * t : 128 * (t + 1)], rhs=kT[1][:],
                start=False, stop=True,
            )
            s_sb = work.tile([128, P], fp32, tag="ssb")
            nc.vector.tensor_copy(s_sb[:], ps_s[:])
            m8 = work.tile([128, 8], fp32, tag="m8")
            nc.vector.max(m8[:], s_sb[:])
            nc.vector.memset(m8[:, TOPK:], -3.0e38)
            zap = work.tile([128, P], fp32, tag="zap")
            nc.vector.match_replace(
                out=zap[:], in_to_replace=m8[:], in_values=s_sb[:], imm_value=-3.0e38
            )
            oh = work.tile([128, P], fp32, tag="oh")
            nc.vector.tensor_tensor(out=oh[:], in0=s_sb[:], in1=zap[:], op=OP.not_equal)
            nc.gpsimd.dma_start(out=t_oh[128 * t : 128 * (t + 1), :], in_=oh[:])

        # ----- motif document-frequency: colsum of (pc > 0) over all P rows -----
        # idf = log((1+P)/(1+sum_motif)) + 1 = C - ln(1 + sum_motif)
        C = float(np.log(np.float64(1.0 + P)) + 1.0)
        idf = res.tile([NB, V], fp32, tag="idf")
        if use_allgather:
            # each core column-sums its own V/8 columns over all P rows, then
            # an AllGather assembles the full [V] document frequency.
            VC = V // NCORES
            ind = [
                res.tile([128, VC], bf16, tag=f"ind{k}", name=f"ind{k}")
                for k in range(4)
            ]
            for k in range(4):
                nc.vector.tensor_scalar(
                    out=ind[k][:], in0=pcts[k][:], scalar1=0.0, scalar2=None,
                    op0=OP.is_gt,
                )
            ones1 = res.tile([128, 1], bf16, tag="ones")
            nc.vector.memset(ones1[:], 1.0)
            ps_c = psc.tile([1, VC], fp32, tag="cs")
            for k in range(4):
                nc.tensor.matmul(
                    out=ps_c[:, :], lhsT=ones1[:], rhs=ind[k][:, :],
                    start=(k == 0), stop=(k == 3),
                )
            cs_sb = res.tile([1, VC], fp32, tag="cssb")
            nc.vector.tensor_copy(cs_sb[:], ps_c[:])
            dram = ctx.enter_context(
                tc.tile_pool(name=f"dram{it}", bufs=1, space="DRAM")
            )
            cs_in = dram.tile([1, VC], fp32, tag="csin")
            cs_out = dram.tile([NCORES, VC], fp32, tag="csout")
            nc.gpsimd.dma_start(cs_in[:], cs_sb[:])
            nc.gpsimd.collective_compute(
                "AllGather",
                mybir.AluOpType.bypass,
                replica_groups=[list(range(NCORES))],
                ins=[cs_in[:].opt()],
                outs=[cs_out[:].opt()],
            )
            csb = res.tile([NB, V], fp32, tag="csb")
            src = cs_out[:].rearrange("a b -> (a b)").unsqueeze(0).to_broadcast((NB, V))
            nc.scalar.dma_start(out=csb[:], in_=src)
            lnf = lnp.tile([NB, V], fp32, tag="lnh")
            nc.scalar.activation(out=lnf[:], in_=csb[:], func=AF.Ln, bias=1.0, scale=1.0)
            nc.vector.tensor_scalar(
                out=idf[:, :], in0=lnf[:], scalar1=C, scalar2=-1.0,
                op0=OP.subtract, op1=OP.mult,
            )
        else:
            ind = [
                res.tile([128, V], bf16, tag=f"ind{k}", name=f"ind{k}")
                for k in range(4)
            ]
            for k in range(4):
                nc.vector.tensor_scalar(
                    out=ind[k][:], in0=pcts[k][:], scalar1=0.0, scalar2=None,
                    op0=OP.is_gt,
                )
            ones64 = res.tile([128, NB], bf16, tag="ones")
            nc.vector.memset(ones64[:], 1.0)
            VH = V // 2
            for half in range(2):
                ps_c = psc.tile([NB, VH], fp32, tag="cs")
                for j in range(4):
                    col0 = half * VH + j * 512
                    for k in range(4):
                        nc.tensor.matmul(
                            out=ps_c[:, j * 512 : (j + 1) * 512],
                            lhsT=ones64[:],
                            rhs=ind[k][:, col0 : col0 + 512],
                            start=(k == 0),
                            stop=(k == 3),
                        )
                lnh = lnp.tile([NB, VH], fp32, tag="lnh")
                nc.scalar.activation(
                    out=lnh[:], in_=ps_c[:], func=AF.Ln, bias=1.0, scale=1.0
                )
                nc.vector.tensor_scalar(
                    out=idf[:, half * VH : (half + 1) * VH],
                    in0=lnh[:], scalar1=C, scalar2=-1.0,
                    op0=OP.subtract, op1=OP.mult,
                )

        # ----- TF part and block = pc_rows/(rowsum+1) * idf -----
        rs = res.tile([NB, 1], fp32, tag="rs")
        nc.vector.reduce_sum(rs[:], pcrt[:], axis=AX.X)
        rsp = res.tile([NB, 1], fp32, tag="rsp")
        nc.vector.tensor_scalar_add(rsp[:], rs[:], 1.0)
        rr = res.tile([NB, 1], fp32, tag="rr")
        nc.vector.reciprocal(rr[:], rsp[:])
        blk = res.tile([NB, V], fp32, tag="blk")
        nc.vector.scalar_tensor_tensor(
            out=blk[:], in0=pcrt[:], scalar=rr[:], in1=idf[:],
            op0=OP.mult, op1=OP.mult,
        )
        nc.gpsimd.dma_start(out=t_blk[:, :], in_=blk[:])

        # ----- node_feat passthrough shards -----
        for t in range(4):
            nc.gpsimd.dma_start(out=t_nfx[128 * t : 128 * (t + 1), :], in_=xt[t][:])
        nc.gpsimd.dma_start(out=t_nfk[:, :], in_=vqst[:])


def _declare_io(nc, fp32, use_allgather, external=True):
    kk = "ExternalOutput" if external else "Internal"
    aps = {
        "xs": nc.dram_tensor("xs", [NX, H], fp32, kind="ExternalInput").ap(),
        "vq": nc.dram_tensor("vq", [P, H], fp32, kind="ExternalInput").ap(),
        "pcr": nc.dram_tensor("pcr", [NB, V], fp32, kind="ExternalInput").ap(),
        "vqs": nc.dram_tensor("vqs", [NB, H], fp32, kind="ExternalInput").ap(),
        "onehot": nc.dram_tensor("onehot", [NX, P], fp32, kind=kk).ap(),
        "block": nc.dram_tensor("block", [NB, V], fp32, kind=kk).ap(),
        "zeros": nc.dram_tensor("zeros", [ZR, DIM], fp32, kind=kk).ap(),
        "nfx": nc.dram_tensor("nfx", [NX, H], fp32, kind=kk).ap(),
        "nfk": nc.dram_tensor("nfk", [NB, H], fp32, kind=kk).ap(),
    }
    if use_allgather:
        aps["pcc"] = nc.dram_tensor(
            "pcc", [P, V // NCORES], fp32, kind="ExternalInput"
        ).ap()
    else:
        aps["pc"] = nc.dram_tensor("pc", [P, V], fp32, kind="ExternalInput").ap()
    return aps


def build_nc(write_zeros=WRITE_ZEROS, zero_bcast=True, repeat=1, use_allgather=False):
    import concourse.bacc as bacc
    import concourse.mybir as mybir
    import concourse.tile as tile

    fp32 = mybir.dt.float32
    nc = bacc.Bacc("TRN2", target_bir_lowering=False, debug=False, num_devices=NCORES)
    aps = _declare_io(nc, fp32, use_allgather, external=True)

    with tile.TileContext(nc) as tc:
        for it in range(repeat):
            _emit(
                tc, aps, it=it, write_zeros=write_zeros, zero_bcast=zero_bcast,
                use_allgather=use_allgather,
            )
    nc.finalize()
    return nc


def build_timing_nc(write_zeros=WRITE_ZEROS, zero_bcast=True, repeat=1,
                    use_allgather=False):
    """Same program but all big outputs go to Internal DRAM and only a tiny
    ExternalOutput is returned — isolates device exec time from host<->device
    transfer when measuring wall-clock deltas between repeat counts."""
    import concourse.bacc as bacc
    import concourse.mybir as mybir
    import concourse.tile as tile

    fp32 = mybir.dt.float32
    nc = bacc.Bacc("TRN2", target_bir_lowering=False, debug=False, num_devices=NCORES)
    aps = _declare_io(nc, fp32, use_allgather, external=False)
    done = nc.dram_tensor("done", [NB, 8], fp32, kind="ExternalOutput").ap()

    with tile.TileContext(nc) as tc:
        for it in range(repeat):
            if it:
                tc.strict_bb_all_engine_barrier()
            _emit(tc, aps, it=it, write_zeros=write_zeros, zero_bcast=zero_bcast,
                  use_allgather=use_allgather)
        with tc.tile_pool(name="donep", bufs=1) as dp:
            dt_ = dp.tile([NB, 8], fp32, tag="dn")
            nc.vector.memset(dt_[:], 1.0)
            nc.sync.dma_start(out=done[:, :], in_=dt_[:])
    nc.finalize()
    return nc


def build_loop_nc(write_zeros=WRITE_ZEROS, zero_bcast=True, iters=8,
                  use_allgather=False):
    """Timing variant: the whole per-core program inside a hardware For_i loop
    (the loop back-edge is a full barrier, so iterations serialize). Big
    outputs go to Internal DRAM; only a tiny tensor is an ExternalOutput."""
    import concourse.bacc as bacc
    import concourse.mybir as mybir
    import concourse.tile as tile

    fp32 = mybir.dt.float32
    nc = bacc.Bacc("TRN2", target_bir_lowering=False, debug=False, num_devices=NCORES)
    aps = _declare_io(nc, fp32, use_allgather, external=False)
    done = nc.dram_tensor("done", [NB, 8], fp32, kind="ExternalOutput").ap()

    with tile.TileContext(nc) as tc:
        with tc.For_i(0, iters, 1):
            _emit(tc, aps, it=0, write_zeros=write_zeros, zero_bcast=zero_bcast,
                  use_allgather=use_allgather)
        with tc.tile_pool(name="donep", bufs=1) as dp:
            dt_ = dp.tile([NB, 8], fp32, tag="dn")
            nc.vector.memset(dt_[:], 1.0)
            nc.sync.dma_start(out=done[:, :], in_=dt_[:])
    nc.finalize()
    return nc


def _get_nc():
    key = (WRITE_ZEROS, USE_ALLGATHER)
    if key not in _NC_CACHE:
        _NC_CACHE[key] = build_nc(
            write_zeros=WRITE_ZEROS, use_allgather=USE_ALLGATHER
        )
    return _NC_CACHE[key]


def make_in_maps(x, vq, pc, use_allgather=False):
    in_maps = []
    vc = V // NCORES
    for c in range(NCORES):
        m = {
            "xs": np.ascontiguousarray(x[NX * c : NX * (c + 1)]),
            "vq": vq,
            "pcr": np.ascontiguousarray(pc[NB * c : NB * (c + 1)]),
            "vqs": np.ascontiguousarray(vq[NB * c : NB * (c + 1)]),
        }
        if use_allgather:
            m["pcc"] = np.ascontiguousarray(pc[:, vc * c : vc * (c + 1)])
        else:
            m["pc"] = pc
        in_maps.append(m)
    return in_maps


def assemble(results):
    adj = np.concatenate([r["zeros"] for r in results], axis=0)
    adj[:N, N : N + P] = np.concatenate([r["onehot"] for r in results], axis=0)
    adj[N : N + P, N + P :] = np.concatenate([r["block"] for r in results], axis=0)
    nf = np.concatenate(
        [r["nfx"] for r in results] + [r["nfk"] for r in results], axis=0
    )
    return adj, nf


def kernel(**inputs):
    from concourse.bass_utils import run_bass_kernel_spmd

    x = np.ascontiguousarray(np.asarray(inputs["x"], dtype=np.float32))
    vq = np.ascontiguousarray(np.asarray(inputs["vq_keys"], dtype=np.float32))
    pc = np.ascontiguousarray(np.asarray(inputs["prototype_count"], dtype=np.float32))
    assert x.shape == (N, H) and vq.shape == (P, H) and pc.shape == (P, V)

    nc = _get_nc()
    in_maps = make_in_maps(x, vq, pc, use_allgather=USE_ALLGATHER)
    res = run_bass_kernel_spmd(nc, in_maps, core_ids=list(range(NCORES))).results
    return assemble(res)


if __name__ == "__main__":
    rng = np.random.default_rng(0)
    x = rng.standard_normal((N, H), dtype=np.float32)
    vq = rng.standard_normal((P, H), dtype=np.float32)
    pc = rng.random((P, V), dtype=np.float32)
    adj, nf = kernel(x=x, vq_keys=vq, prototype_count=pc)
    print("adj", adj.shape, "nnz", int((adj != 0).sum()), "nf", nf.shape)


# revision 36
# speedup vs baseline: 1.1305x; 1.1305x over previous
"""Trainium2 Bass kernel: top-k cosine retrieval adjacency + TF-IDF block.

Builds the dense (N+P+V)^2 adjacency and node features of the reference
nn.Module on 8 NeuronCores.

Work decomposition (SPMD, identical program on all 8 cores, data-parallel):
  - core c computes the top-4 one-hot rows for x rows [512c, 512(c+1))
  - core c computes TF-IDF block rows [64c, 64(c+1))
  - core c writes 1/8 of the zero canvas rows of the adjacency
  - node_feat is a passthrough copy, sharded the same way
Host side only re-assembles the row shards (np.concatenate) into the
full [8704, 8704] adjacency and [4608, 256] node features.
"""

import numpy as np

N, P, V, H, TOPK = 4096, 512, 4096, 256, 4
DIM = N + P + V            # 8704
NCORES = 8
NX = N // NCORES           # 512 x rows per core
NB = P // NCORES           # 64 prototype rows per core
ZR = DIM // NCORES         # 1088 canvas rows per core

# device writes the full zero canvas (memory-roofline path). If False the
# canvas relies on the runtime's zero-initialized output buffers.
WRITE_ZEROS = True
# shard the document-frequency column sums across cores + AllGather instead of
# every core reading the full replicated prototype_count.
USE_ALLGATHER = False

_NC_CACHE = {}


def _emit(tc, aps, it=0, write_zeros=True, zero_bcast=True, use_allgather=False):
    """Emit the per-core program into TileContext tc."""
    from contextlib import ExitStack

    import concourse.mybir as mybir
    from concourse.masks import make_identity

    nc = tc.nc
    fp32 = mybir.dt.float32
    bf16 = mybir.dt.bfloat16
    AF = mybir.ActivationFunctionType
    OP = mybir.AluOpType
    AX = mybir.AxisListType

    t_xs = aps["xs"]
    t_vq = aps["vq"]
    t_pc = aps.get("pc")
    t_pcr = aps.get("pcr")
    t_vqs = aps["vqs"]
    t_ohrow = aps["ohrow"]
    t_blkrow = aps["blkrow"]
    t_zrow = aps["zrow"]
    t_nfx = aps["nfx"]
    t_nfk = aps["nfk"]

    with ExitStack() as ctx:
        # the full-width zero tile (zero_bcast=False) needs the SBUF that the
        # 4-deep pc streaming pool otherwise uses
        iop_bufs = 4 if (use_allgather or zero_bcast) else 2
        res = ctx.enter_context(tc.tile_pool(name=f"res{it}", bufs=1))
        iop = ctx.enter_context(tc.tile_pool(name=f"iop{it}", bufs=iop_bufs))
        work = ctx.enter_context(tc.tile_pool(name=f"work{it}", bufs=2))
        lnp = ctx.enter_context(tc.tile_pool(name=f"lnp{it}", bufs=1))
        pst = ctx.enter_context(tc.tile_pool(name=f"pst{it}", bufs=2, space="PSUM"))
        pss = ctx.enter_context(tc.tile_pool(name=f"pss{it}", bufs=2, space="PSUM"))
        psc = ctx.enter_context(tc.tile_pool(name=f"psc{it}", bufs=1, space="PSUM"))

        # ----- input loads first so the compute chain starts immediately and
        # its (small) stores hide under the bulk zero-canvas writes -----
        xt, vqt, knt = [], [], []
        for t in range(4):
            xtile = res.tile([128, H], fp32, tag=f"x{t}")
            nc.scalar.dma_start(out=xtile[:], in_=t_xs[128 * t : 128 * (t + 1), :])
            xt.append(xtile)
        for t in range(4):
            vtile = res.tile([128, H], fp32, tag=f"vq{t}")
            nc.scalar.dma_start(out=vtile[:], in_=t_vq[128 * t : 128 * (t + 1), :])
            vqt.append(vtile)
        vqst = res.tile([NB, H], fp32, tag="vqs")
        nc.scalar.dma_start(out=vqst[:], in_=t_vqs[:, :])
        pcts = []
        if use_allgather:
            # per-core column slice of prototype_count: [P, V/NCORES]
            t_pcc = aps["pcc"]
            for k in range(4):
                pct = iop.tile([128, V // NCORES], fp32, tag="pc", name=f"pct{k}")
                nc.scalar.dma_start(out=pct[:], in_=t_pcc[128 * k : 128 * (k + 1), :])
                pcts.append(pct)
            pcrt = res.tile([NB, V], fp32, tag="pcr")
            nc.scalar.dma_start(out=pcrt[:], in_=t_pcr[:, :])
        else:
            # host passes pc row-rotated so this core's 64 block rows sit at
            # rows 0..63 (column sums are row-order invariant and exact), and
            # the block chain reuses the already-loaded tile — no extra read.
            for k in range(4):
                pct = iop.tile([128, V], fp32, tag="pc", name=f"pct{k}")
                nc.scalar.dma_start(out=pct[:], in_=t_pc[128 * k : 128 * (k + 1), :])
                pcts.append(pct)
            pcrt = pcts[0]

        # ----- bulk zero writes: full-width adjacency row shards, composed as
        # column segments so data regions are written exactly once -----
        if write_zeros:
            zt = res.tile([128, 512], fp32, tag="zt")
            nc.vector.memset(zt[:], 0.0)

            def zseg(eng, out_ap, rows, cols):
                src = zt[:rows, :].unsqueeze(1).to_broadcast((rows, cols // 512, 512))
                eng.dma_start(out=out_ap, in_=src)

            # zrow: NX fully-zero rows per core
            for t in range(4):
                eng = nc.sync if t < 3 else nc.scalar
                zseg(eng, t_zrow[128 * t : 128 * (t + 1), :], 128, DIM)
            # ohrow: zero cols 0:N and N+P:DIM (onehot cols N:N+P written later)
            for t in range(4):
                ea = nc.sync if t < 2 else nc.scalar
                ec = nc.scalar if t < 2 else nc.sync
                zseg(ea, t_ohrow[128 * t : 128 * (t + 1), 0:N], 128, N)
                zseg(ec, t_ohrow[128 * t : 128 * (t + 1), N + P : DIM], 128, DIM - N - P)
            # blkrow: zero cols 0:N+P (block cols N+P:DIM written later)
            zseg(nc.sync, t_blkrow[:, 0 : N + P], NB, N + P)

        # ----- normalize keys -----
        for t in range(4):
            sq = work.tile([128, H], fp32, tag="sq")
            ss = res.tile([128, 1], fp32, tag=f"ss{t}")
            nc.scalar.activation(out=sq[:], in_=vqt[t][:], func=AF.Square, accum_out=ss[:])
            nrm = res.tile([128, 1], fp32, tag=f"nrm{t}")
            nc.scalar.sqrt(nrm[:], ss[:])
            rn = res.tile([128, 1], fp32, tag=f"rn{t}")
            nc.vector.reciprocal(rn[:], nrm[:])
            kt = res.tile([128, H], fp32, tag=f"kn{t}")
            nc.vector.tensor_scalar_mul(kt[:], vqt[t][:], rn[:])
            knt.append(kt)

        # ----- PE transposes: x^T [H, NX], kn^T [H, P] -----
        ident = res.tile([128, 128], fp32, tag="ident")
        make_identity(nc, ident[:])
        # warmup: advance PE's observed clock past the gpsimd identity write so
        # real transposes carry a single semaphore wait (walrus LDW limit).
        warm = pst.tile([128, 128], fp32, tag="tp", name="warm")
        nc.tensor.transpose(warm[:], ident[:], ident[:])
        xT = [res.tile([128, NX], fp32, tag=f"xT{h}", name=f"xT{h}") for h in range(2)]
        kT = [res.tile([128, P], fp32, tag=f"kT{h}", name=f"kT{h}") for h in range(2)]
        for t in range(4):
            for h in range(2):
                ps = pst.tile([128, 128], fp32, tag="tp")
                nc.tensor.transpose(ps[:], xt[t][:, 128 * h : 128 * (h + 1)], ident[:])
                nc.vector.tensor_copy(xT[h][:, 128 * t : 128 * (t + 1)], ps[:])
                ps2 = pst.tile([128, 128], fp32, tag="tp")
                nc.tensor.transpose(ps2[:], knt[t][:, 128 * h : 128 * (h + 1)], ident[:])
                nc.vector.tensor_copy(kT[h][:, 128 * t : 128 * (t + 1)], ps2[:])

        # ----- s = x @ kn^T (f32), top-4 -> one-hot rows -----
        for t in range(4):
            ps_s = pss.tile([128, P], fp32, tag="s")
            nc.tensor.matmul(
                out=ps_s[:], lhsT=xT[0][:, 128 * t : 128 * (t + 1)], rhs=kT[0][:],
                start=True, stop=False,
            )
            nc.tensor.matmul(
                out=ps_s[:], lhsT=xT[1][:, 128 * t : 128 * (t + 1)], rhs=kT[1][:],
                start=False, stop=True,
            )
            s_sb = work.tile([128, P], fp32, tag="ssb")
            nc.vector.tensor_copy(s_sb[:], ps_s[:])
            m8 = work.tile([128, 8], fp32, tag="m8")
            nc.vector.max(m8[:], s_sb[:])
            nc.vector.memset(m8[:, TOPK:], -3.0e38)
            zap = work.tile([128, P], fp32, tag="zap")
            nc.vector.match_replace(
                out=zap[:], in_to_replace=m8[:], in_values=s_sb[:], imm_value=-3.0e38
            )
            oh = work.tile([128, P], fp32, tag="oh")
            nc.vector.tensor_tensor(out=oh[:], in0=s_sb[:], in1=zap[:], op=OP.not_equal)
            nc.gpsimd.dma_start(out=t_ohrow[128 * t : 128 * (t + 1), N : N + P], in_=oh[:])

        # ----- motif document-frequency: colsum of (pc > 0) over all P rows -----
        # idf = log((1+P)/(1+sum_motif)) + 1 = C - ln(1 + sum_motif)
        C = float(np.log(np.float64(1.0 + P)) + 1.0)
        idf = res.tile([NB, V], fp32, tag="idf")
        if use_allgather:
            # each core column-sums its own V/8 columns over all P rows, then
            # an AllGather assembles the full [V] document frequency.
            VC = V // NCORES
            ind = [
                res.tile([128, VC], bf16, tag=f"ind{k}", name=f"ind{k}")
                for k in range(4)
            ]
            for k in range(4):
                nc.vector.tensor_scalar(
                    out=ind[k][:], in0=pcts[k][:], scalar1=0.0, scalar2=None,
                    op0=OP.is_gt,
                )
            ones1 = res.tile([128, 1], bf16, tag="ones")
            nc.vector.memset(ones1[:], 1.0)
            ps_c = psc.tile([1, VC], fp32, tag="cs")
            for k in range(4):
                nc.tensor.matmul(
                    out=ps_c[:, :], lhsT=ones1[:], rhs=ind[k][:, :],
                    start=(k == 0), stop=(k == 3),
                )
            cs_sb = res.tile([1, VC], fp32, tag="cssb")
            nc.vector.tensor_copy(cs_sb[:], ps_c[:])
            dram = ctx.enter_context(
                tc.tile_pool(name=f"dram{it}", bufs=1, space="DRAM")
            )
            cs_in = dram.tile([1, VC], fp32, tag="csin")
            cs_out = dram.tile([NCORES, VC], fp32, tag="csout")
            nc.gpsimd.dma_start(cs_in[:], cs_sb[:])
            nc.gpsimd.collective_compute(
                "AllGather",
                mybir.AluOpType.bypass,
                replica_groups=[list(range(NCORES))],
                ins=[cs_in[:].opt()],
                outs=[cs_out[:].opt()],
            )
            csb = res.tile([NB, V], fp32, tag="csb")
            src = cs_out[:].rearrange("a b -> (a b)").unsqueeze(0).to_broadcast((NB, V))
            nc.scalar.dma_start(out=csb[:], in_=src)
            lnf = lnp.tile([NB, V], fp32, tag="lnh")
            nc.scalar.activation(out=lnf[:], in_=csb[:], func=AF.Ln, bias=1.0, scale=1.0)
            nc.vector.tensor_scalar(
                out=idf[:, :], in0=lnf[:], scalar1=C, scalar2=-1.0,
                op0=OP.subtract, op1=OP.mult,
            )
        else:
            ind = [
                res.tile([128, V], bf16, tag=f"ind{k}", name=f"ind{k}")
                for k in range(4)
            ]
            for k in range(4):
                nc.vector.tensor_scalar(
                    out=ind[k][:], in0=pcts[k][:], scalar1=0.0, scalar2=None,
                    op0=OP.is_gt,
                )
            ones64 = res.tile([128, NB], bf16, tag="ones")
            nc.vector.memset(ones64[:], 1.0)
            VH = V // 2
            for half in range(2):
                ps_c = psc.tile([NB, VH], fp32, tag="cs")
                for j in range(4):
                    col0 = half * VH + j * 512
                    for k in range(4):
                        nc.tensor.matmul(
                            out=ps_c[:, j * 512 : (j + 1) * 512],
                            lhsT=ones64[:],
                            rhs=ind[k][:, col0 : col0 + 512],
                            start=(k == 0),
                            stop=(k == 3),
                        )
                lnh = lnp.tile([NB, VH], fp32, tag="lnh")
                nc.scalar.activation(
                    out=lnh[:], in_=ps_c[:], func=AF.Ln, bias=1.0, scale=1.0
                )
                nc.vector.tensor_scalar(
                    out=idf[:, half * VH : (half + 1) * VH],
                    in0=lnh[:], scalar1=C, scalar2=-1.0,
                    op0=OP.subtract, op1=OP.mult,
                )

        # ----- TF part and block = pc_rows/(rowsum+1) * idf -----
        rs = res.tile([NB, 1], fp32, tag="rs")
        nc.vector.reduce_sum(rs[:], pcrt[:NB, :], axis=AX.X)
        rsp = res.tile([NB, 1], fp32, tag="rsp")
        nc.vector.tensor_scalar_add(rsp[:], rs[:], 1.0)
        rr = res.tile([NB, 1], fp32, tag="rr")
        nc.vector.reciprocal(rr[:], rsp[:])
        blk = res.tile([NB, V], fp32, tag="blk")
        nc.vector.scalar_tensor_tensor(
            out=blk[:], in0=pcrt[:NB, :], scalar=rr[:], in1=idf[:],
            op0=OP.mult, op1=OP.mult,
        )
        nc.gpsimd.dma_start(out=t_blkrow[:, N + P : DIM], in_=blk[:])

        # ----- node_feat passthrough shards -----
        for t in range(4):
            nc.gpsimd.dma_start(out=t_nfx[128 * t : 128 * (t + 1), :], in_=xt[t][:])
        nc.gpsimd.dma_start(out=t_nfk[:, :], in_=vqst[:])


def _declare_io(nc, fp32, use_allgather, external=True):
    kk = "ExternalOutput" if external else "Internal"
    aps = {
        "xs": nc.dram_tensor("xs", [NX, H], fp32, kind="ExternalInput").ap(),
        "vq": nc.dram_tensor("vq", [P, H], fp32, kind="ExternalInput").ap(),
        "vqs": nc.dram_tensor("vqs", [NB, H], fp32, kind="ExternalInput").ap(),
        "ohrow": nc.dram_tensor("ohrow", [NX, DIM], fp32, kind=kk).ap(),
        "blkrow": nc.dram_tensor("blkrow", [NB, DIM], fp32, kind=kk).ap(),
        "zrow": nc.dram_tensor("zrow", [NX, DIM], fp32, kind=kk).ap(),
        "nfx": nc.dram_tensor("nfx", [NX, H], fp32, kind=kk).ap(),
        "nfk": nc.dram_tensor("nfk", [NB, H], fp32, kind=kk).ap(),
    }
    if use_allgather:
        aps["pcc"] = nc.dram_tensor(
            "pcc", [P, V // NCORES], fp32, kind="ExternalInput"
        ).ap()
        aps["pcr"] = nc.dram_tensor("pcr", [NB, V], fp32, kind="ExternalInput").ap()
    else:
        aps["pc"] = nc.dram_tensor("pc", [P, V], fp32, kind="ExternalInput").ap()
    return aps


def build_nc(write_zeros=WRITE_ZEROS, zero_bcast=True, repeat=1, use_allgather=False):
    import concourse.bacc as bacc
    import concourse.mybir as mybir
    import concourse.tile as tile

    fp32 = mybir.dt.float32
    nc = bacc.Bacc("TRN2", target_bir_lowering=False, debug=False, num_devices=NCORES)
    aps = _declare_io(nc, fp32, use_allgather, external=True)

    with tile.TileContext(nc) as tc:
        for it in range(repeat):
            _emit(
                tc, aps, it=it, write_zeros=write_zeros, zero_bcast=zero_bcast,
                use_allgather=use_allgather,
            )
    nc.finalize()
    return nc


def build_timing_nc(write_zeros=WRITE_ZEROS, zero_bcast=True, repeat=1,
                    use_allgather=False):
    """Same program but all big outputs go to Internal DRAM and only a tiny
    ExternalOutput is returned — isolates device exec time from host<->device
    transfer when measuring wall-clock deltas between repeat counts."""
    import concourse.bacc as bacc
    import concourse.mybir as mybir
    import concourse.tile as tile

    fp32 = mybir.dt.float32
    nc = bacc.Bacc("TRN2", target_bir_lowering=False, debug=False, num_devices=NCORES)
    aps = _declare_io(nc, fp32, use_allgather, external=False)
    done = nc.dram_tensor("done", [NB, 8], fp32, kind="ExternalOutput").ap()

    with tile.TileContext(nc) as tc:
        for it in range(repeat):
            if it:
                tc.strict_bb_all_engine_barrier()
            _emit(tc, aps, it=it, write_zeros=write_zeros, zero_bcast=zero_bcast,
                  use_allgather=use_allgather)
        with tc.tile_pool(name="donep", bufs=1) as dp:
            dt_ = dp.tile([NB, 8], fp32, tag="dn")
            nc.vector.memset(dt_[:], 1.0)
            nc.sync.dma_start(out=done[:, :], in_=dt_[:])
    nc.finalize()
    return nc


def build_loop_nc(write_zeros=WRITE_ZEROS, zero_bcast=True, iters=8,
                  use_allgather=False):
    """Timing variant: the whole per-core program inside a hardware For_i loop
    (the loop back-edge is a full barrier, so iterations serialize). Big
    outputs go to Internal DRAM; only a tiny tensor is an ExternalOutput."""
    import concourse.bacc as bacc
    import concourse.mybir as mybir
    import concourse.tile as tile

    fp32 = mybir.dt.float32
    nc = bacc.Bacc("TRN2", target_bir_lowering=False, debug=False, num_devices=NCORES)
    aps = _declare_io(nc, fp32, use_allgather, external=False)
    done = nc.dram_tensor("done", [NB, 8], fp32, kind="ExternalOutput").ap()

    with tile.TileContext(nc) as tc:
        with tc.For_i(0, iters, 1):
            _emit(tc, aps, it=0, write_zeros=write_zeros, zero_bcast=zero_bcast,
                  use_allgather=use_allgather)
        with tc.tile_pool(name="donep", bufs=1) as dp:
            dt_ = dp.tile([NB, 8], fp32, tag="dn")
            nc.vector.memset(dt_[:], 1.0)
            nc.sync.dma_start(out=done[:, :], in_=dt_[:])
    nc.finalize()
    return nc


def _get_nc():
    key = (WRITE_ZEROS, USE_ALLGATHER)
    if key not in _NC_CACHE:
        _NC_CACHE[key] = build_nc(
            write_zeros=WRITE_ZEROS, use_allgather=USE_ALLGATHER
        )
    return _NC_CACHE[key]


def make_in_maps(x, vq, pc, use_allgather=False):
    in_maps = []
    vc = V // NCORES
    for c in range(NCORES):
        m = {
            "xs": np.ascontiguousarray(x[NX * c : NX * (c + 1)]),
            "vq": vq,
            "vqs": np.ascontiguousarray(vq[NB * c : NB * (c + 1)]),
        }
        if use_allgather:
            m["pcc"] = np.ascontiguousarray(pc[:, vc * c : vc * (c + 1)])
            m["pcr"] = np.ascontiguousarray(pc[NB * c : NB * (c + 1)])
        else:
            # row-rotated so this core's block rows land at rows 0..63; the
            # column sums over all rows are unaffected (order-invariant).
            m["pc"] = np.ascontiguousarray(np.roll(pc, -NB * c, axis=0))
        in_maps.append(m)
    return in_maps


def assemble(results):
    adj = np.concatenate(
        [r["ohrow"] for r in results]
        + [r["blkrow"] for r in results]
        + [r["zrow"] for r in results],
        axis=0,
    )
    nf = np.concatenate(
        [r["nfx"] for r in results] + [r["nfk"] for r in results], axis=0
    )
    return adj, nf


def kernel(**inputs):
    from concourse.bass_utils import run_bass_kernel_spmd

    x = np.ascontiguousarray(np.asarray(inputs["x"], dtype=np.float32))
    vq = np.ascontiguousarray(np.asarray(inputs["vq_keys"], dtype=np.float32))
    pc = np.ascontiguousarray(np.asarray(inputs["prototype_count"], dtype=np.float32))
    assert x.shape == (N, H) and vq.shape == (P, H) and pc.shape == (P, V)

    nc = _get_nc()
    in_maps = make_in_maps(x, vq, pc, use_allgather=USE_ALLGATHER)
    res = run_bass_kernel_spmd(nc, in_maps, core_ids=list(range(NCORES))).results
    return assemble(res)


if __name__ == "__main__":
    rng = np.random.default_rng(0)
    x = rng.standard_normal((N, H), dtype=np.float32)
    vq = rng.standard_normal((P, H), dtype=np.float32)
    pc = rng.random((P, V), dtype=np.float32)
    adj, nf = kernel(x=x, vq_keys=vq, prototype_count=pc)
    print("adj", adj.shape, "nnz", int((adj != 0).sum()), "nf", nf.shape)
